# revision 9
# baseline (speedup 1.0000x reference)
"""Josephson-junction SDE: Euler-Maruyama fully on-device, batch-sharded 8 ways.

Per core: batch 2048, state packed as [128 part, 32 cols] (= 2048 batch x 2
components on free-dim halves).  Per step t (f32 throughout):

  M1  (STT[64]):  [PHI^{t+1} | X'] = ([V | S'] * dt) + [PHI | Z'']
  Q   (TS [32]):  q = i32( PHI^{t+1} * 1/2pi )          (round-to-nearest cast)
  R   (STT[32]):  r = (q * -2pi) + PHI^{t+1}            (range-reduced angle)
  D   (TT [32]):  [d | -d] = PHIswap - PHI              (swap via strided AP)
  P1,P2(STT[16]): p_c = (V_c * a_c) + X'_c
  NV  (STT[32]):  V^{t+1} = (D * dt*kappa) + P          (kappa1==kappa2 path)
  ACT:            S'^{t+1} = Sin(-r)  ( = -sin(phi^{t+1}) )

with a_c = 1 - dt*beta_c and host-prefolded noise Z'' = sigma_c*sqrt(dt)*z + dt*i_c.
Trajectory slots stream to DRAM in 128-step blocks; noise streams in the same way.
"""

import numpy as np

import concourse.bass as bass
import concourse.mybir as mybir
from concourse import bass_utils
from concourse.ap import AP

F32 = mybir.dt.float32
I32 = mybir.dt.int32
ALU = mybir.AluOpType
ACTF = mybir.ActivationFunctionType

N_CORES = 8
RS = 256          # phi/v/z ring slots (32 cols each)
KB = 128          # steps per DMA block (out and in)
SMALL = 4         # small ring slots for s'/x'/p/d/r/q

_CACHE = {}
LAST_RESULTS = None

TWO_PI = float(2.0 * np.pi)
INV_2PI = float(1.0 / (2.0 * np.pi))
RND_C = 12582912.0  # 1.5 * 2**23: (x + C) - C rounds x to nearest int in f32


def _build(dt, a1, a2, kp1, kp2, n_steps):
    """n_steps = N-1 update steps; emits the full unrolled program."""
    dt = float(dt)
    kap_equal = (kp1 == kp2)
    nc = bass.Bass()
    y0_in = nc.dram_tensor("y0_in", [128, 64], F32, kind="ExternalInput")
    z_in = nc.dram_tensor("z_in", [128, n_steps * 32], F32, kind="ExternalInput")
    phi_out = nc.dram_tensor("phi_out", [128, n_steps * 32], F32, kind="ExternalOutput")
    v_out = nc.dram_tensor("v_out", [128, n_steps * 32], F32, kind="ExternalOutput")

    n_blocks = (n_steps + KB - 1) // KB

    # ws column map (f32):
    PHI0 = 0                  # ring: RS*32
    V0 = PHI0 + RS * 32       # ring: RS*32
    Z0 = V0 + RS * 32         # ring: RS*32  (2 blocks of KB in flight)
    S0 = Z0 + RS * 32         # s' ring SMALL*32
    X0 = S0 + SMALL * 32      # x' ring
    P0 = X0 + SMALL * 32      # p ring
    D0 = P0 + SMALL * 32      # d ring
    R0 = D0 + SMALL * 32      # r ring
    Q0 = R0 + SMALL * 32      # rounded-q ring (f32)
    WCOLS = Q0 + SMALL * 32
    ROWS = WCOLS              # ws partition stride (elements)

    def wap(off, w):
        return AP(None, 0, [])  # placeholder; replaced below via closure

    with (
        nc.sbuf_tensor("ws", [128, WCOLS], F32) as ws,
        nc.semaphore("SY") as SY,
        nc.semaphore("SZ") as SZ,
        nc.semaphore("SV") as SV,
        nc.semaphore("SA") as SA,
        nc.semaphore("SOP") as SOP,
        nc.semaphore("SOV") as SOV,
        nc.Block() as block,
    ):
        def w1(off, w=32):
            # simple [128, w] view at column offset
            return AP(ws, off, [[ROWS, 128], [1, w]])

        def w2(off_a, off_b, w=32):
            # two-segment [128, 2, w] view: cols [off_a:off_a+w] ++ [off_b:off_b+w]
            return AP(ws, off_a, [[ROWS, 128], [off_b - off_a, 2], [1, w]])

        def wswap(off):
            # [128, 2, 16] reading halves swapped: [off+16:off+32] ++ [off:off+16]
            return AP(ws, off + 16, [[ROWS, 128], [-16, 2], [1, 16]])

        def w2d(off, w=32):
            # [128, 2, w] plain adjacent view (free-dims match the swap shape)
            return AP(ws, off, [[ROWS, 128], [w // 2, 2], [1, w // 2]])

        phi_at = lambda t: PHI0 + ((t - 1) % RS) * 32   # PHI^t lives at ring pos (t-1)%RS
        v_at = lambda t: V0 + ((t - 1) % RS) * 32
        z_at = lambda t: Z0 + (t % RS) * 32
        s_at = lambda t: S0 + (t % SMALL) * 32
        x_at = lambda t: X0 + (t % SMALL) * 32
        p_at = lambda t: P0 + (t % SMALL) * 32
        d_at = lambda t: D0 + (t % SMALL) * 32
        r_at = lambda t: R0 + (t % SMALL) * 32
        q_at = lambda t: Q0 + (t % SMALL) * 32

        @block.sync
        def _(sync):
            # prologue: y0 into ring slot for t=0, first two z blocks
            sync.dma_start(w1(phi_at(0)), y0_in[:, 0:32]).then_inc(SY, 16)
            sync.dma_start(w1(v_at(0)), y0_in[:, 32:64]).then_inc(SY, 16)
            for b in range(min(2, n_blocks)):
                lo, hi = b * KB, min(n_steps, (b + 1) * KB)
                sync.dma_start(
                    w1(Z0 + (lo % RS) * 32, (hi - lo) * 32),
                    z_in[:, lo * 32 : hi * 32],
                ).then_inc(SZ, 16)
            # streamed z loads + trajectory stores
            for b in range(2, n_blocks + 2):
                # store block b-2 once its last step retired
                sb = b - 2
                lo, hi = sb * KB, min(n_steps, (sb + 1) * KB)
                tE = hi - 1
                sync.wait_ge(SV, tE + 2)
                sync.dma_start(
                    phi_out[:, lo * 32 : hi * 32], w1(PHI0 + (lo % RS) * 32, (hi - lo) * 32)
                ).then_inc(SOP, 16)
                sync.dma_start(
                    v_out[:, lo * 32 : hi * 32], w1(V0 + (lo % RS) * 32, (hi - lo) * 32)
                ).then_inc(SOV, 16)
                if b < n_blocks:
                    lo2, hi2 = b * KB, min(n_steps, (b + 1) * KB)
                    # ring half reused by block b was consumed by block b-2's steps
                    sync.dma_start(
                        w1(Z0 + (lo2 % RS) * 32, (hi2 - lo2) * 32),
                        z_in[:, lo2 * 32 : hi2 * 32],
                    ).then_inc(SZ, 16)
            sync.wait_ge(SOP, 16 * n_blocks)
            sync.wait_ge(SOV, 16 * n_blocks)

        @block.vector
        def _(vector):
            vector.wait_ge(SY, 32)
            # prologue: q^0, r^0  (round-to-nearest via +C/-C trick, all f32)
            vector.tensor_scalar(w1(r_at(0)), w1(phi_at(0)), INV_2PI, RND_C, ALU.mult, ALU.add)
            vector.drain()
            vector.tensor_scalar(w1(q_at(0)), w1(r_at(0)), RND_C, None, ALU.subtract)
            vector.drain()
            vector.scalar_tensor_tensor(
                w1(r_at(0)), w1(q_at(0)), -TWO_PI, w1(phi_at(0)), ALU.mult, ALU.add
            )
            # drain before inc: make writes visible before signaling
            vector.drain().then_inc(SV, 1)
            for t in range(n_steps):
                if t % KB == 0:
                    vector.wait_ge(SZ, 16 * (t // KB + 1))
                    if t >= RS:
                        # about to overwrite ring block (t//KB - 2); ensure drained
                        vector.wait_ge(SOP, 16 * (t // KB - 1))
                        vector.wait_ge(SOV, 16 * (t // KB - 1))
                vector.wait_ge(SA, t + 1)
                # Order gives every same-engine RAW pair >=1 intervening op
                # (DVE pipeline: a back-to-back consumer reads stale SBUF).
                # M1: [PHI^{t+1} | X'] = ([V^t | S'^t] * dt) + [PHI^t | Z''^t]
                vector.scalar_tensor_tensor(
                    w2(PHI0 + (t % RS) * 32, x_at(t)),
                    w2(v_at(t), s_at(t)),
                    dt,
                    w2(phi_at(t), z_at(t)),
                    ALU.mult,
                    ALU.add,
                )
                # D: [d | -d] = PHIswap^t - PHI^t   (independent of M1)
                vector.tensor_tensor(
                    w2d(d_at(t)), wswap(phi_at(t)), w2d(phi_at(t)), ALU.subtract
                )
                last = t == n_steps - 1
                if not last:
                    # QF = PHI^{t+1}*inv2pi + C   (gap-1 after M1)
                    vector.tensor_scalar(
                        w1(r_at(t + 1)), w1(PHI0 + (t % RS) * 32), INV_2PI, RND_C,
                        ALU.mult, ALU.add,
                    )
                # P1 (gap-2 after M1's X' write)
                vector.scalar_tensor_tensor(
                    w1(p_at(t), 16), w1(v_at(t), 16), a1, w1(x_at(t), 16),
                    ALU.mult, ALU.add,
                )
                if not last:
                    # Q = QF - C   (gap-1 after QF)
                    vector.tensor_scalar(
                        w1(q_at(t + 1)), w1(r_at(t + 1)), RND_C, None, ALU.subtract
                    )
                # P2
                vector.scalar_tensor_tensor(
                    w1(p_at(t) + 16, 16), w1(v_at(t) + 16, 16), a2, w1(x_at(t) + 16, 16),
                    ALU.mult, ALU.add,
                )
                if not last:
                    # R = Q*-2pi + PHI^{t+1}   (gap-1 after Q)
                    vector.scalar_tensor_tensor(
                        w1(r_at(t + 1)),
                        w1(q_at(t + 1)),
                        -TWO_PI,
                        w1(PHI0 + (t % RS) * 32),
                        ALU.mult,
                        ALU.add,
                    )
                # NV (>=2 ops after D/P1/P2)
                if kap_equal:
                    vector.scalar_tensor_tensor(
                        w1(V0 + (t % RS) * 32), w1(d_at(t)), dt * kp1, w1(p_at(t)),
                        ALU.mult, ALU.add,
                    )
                else:
                    vector.scalar_tensor_tensor(
                        w1(V0 + (t % RS) * 32, 16), w1(d_at(t), 16), dt * kp1,
                        w1(p_at(t), 16), ALU.mult, ALU.add,
                    )
                    vector.scalar_tensor_tensor(
                        w1(V0 + (t % RS) * 32 + 16, 16), w1(d_at(t) + 16, 16), dt * kp2,
                        w1(p_at(t) + 16, 16), ALU.mult, ALU.add,
                    )
                vector.drain().then_inc(SV, 1)

        @block.scalar
        def _(scalar):
            for k in range(n_steps):
                scalar.wait_ge(SV, k + 1)
                scalar.activation(w1(s_at(k)), w1(r_at(k)), ACTF.Sin, scale=-1.0)
                scalar.drain().then_inc(SA, 1)

    return nc


def _prep_inputs(params, y0, noise, dt):
    """Host-side: shard + relayout.  Returns per-core in_maps."""
    B = y0.shape[0]
    bpc = B // N_CORES
    n_steps = noise.shape[0]
    sq = np.float32(np.sqrt(dt))
    sig = params[6:8].astype(np.float32)
    drive = params[2:4].astype(np.float32)
    scale = (sig * sq).reshape(1, 1, 2, 1)          # per comp
    off = (drive * np.float32(dt)).reshape(1, 1, 2, 1)

    in_maps = []
    for c in range(N_CORES):
        sl = slice(c * bpc, (c + 1) * bpc)
        # z'': [n_steps, bpc, 2] -> [128, n_steps*32] with cols t*32 + comp*16 + j
        zc = noise[:, sl, :].reshape(n_steps, 128, 16, 2).transpose(1, 0, 3, 2)
        zc = zc * scale + off                        # [128, n_steps, 2, 16]
        zc = np.ascontiguousarray(zc.reshape(128, n_steps * 32), dtype=np.float32)
        y0c = y0[sl].reshape(128, 16, 4)
        y0lay = np.empty((128, 64), np.float32)
        y0lay[:, 0:16] = y0c[:, :, 0]    # phi1
        y0lay[:, 16:32] = y0c[:, :, 2]   # phi2
        y0lay[:, 32:48] = y0c[:, :, 1]   # v1
        y0lay[:, 48:64] = y0c[:, :, 3]   # v2
        in_maps.append({"y0_in": y0lay, "z_in": zc})
    return in_maps


def _assemble(results, y0, n_steps):
    B = y0.shape[0]
    bpc = B // N_CORES
    out = np.empty((B, n_steps + 1, 4), np.float32)
    out[:, 0, :] = y0
    for c in range(N_CORES):
        sl = slice(c * bpc, (c + 1) * bpc)
        ph = results[c]["phi_out"].reshape(128, n_steps, 2, 16)
        vv = results[c]["v_out"].reshape(128, n_steps, 2, 16)
        # batch b = p*16 + j ; comp axis -> state cols
        blk = out[sl, 1:, :].reshape(128, 16, n_steps, 4)
        blk[:, :, :, 0] = ph[:, :, 0, :].transpose(0, 2, 1)
        blk[:, :, :, 2] = ph[:, :, 1, :].transpose(0, 2, 1)
        blk[:, :, :, 1] = vv[:, :, 0, :].transpose(0, 2, 1)
        blk[:, :, :, 3] = vv[:, :, 1, :].transpose(0, 2, 1)
        out[sl, 1:, :] = blk.reshape(bpc, n_steps, 4)
    return out


def kernel(params, y0, noise, T, N):
    global LAST_RESULTS
    params = np.asarray(params, dtype=np.float32)
    y0 = np.asarray(y0, dtype=np.float32)
    noise = np.asarray(noise, dtype=np.float32)
    N = int(N)
    n_steps = N - 1
    assert noise.shape[0] == n_steps
    dt = np.float32(T) / np.float32(N - 1)
    beta1, beta2 = float(params[0]), float(params[1])
    kp1, kp2 = float(params[4]), float(params[5])
    a1 = float(np.float32(1.0) - dt * np.float32(beta1))
    a2 = float(np.float32(1.0) - dt * np.float32(beta2))

    key = (float(dt), a1, a2, kp1, kp2, n_steps)
    if key not in _CACHE:
        _CACHE[key] = _build(dt, a1, a2, kp1, kp2, n_steps)
    nc = _CACHE[key]

    in_maps = _prep_inputs(params, y0, noise, float(dt))
    res = bass_utils.run_bass_kernel_spmd(nc, in_maps, core_ids=list(range(N_CORES)))
    LAST_RESULTS = res
    return _assemble(res.results, y0, n_steps)


# revision 11
# speedup vs baseline: 1.0422x; 1.0422x over previous
"""Josephson-junction SDE: Euler-Maruyama fully on-device, batch-sharded 8 ways.

Per core: batch 2048, state packed as [128 part, 32 cols] (= 2048 batch x 2
components on free-dim halves).  Per step t (f32 throughout):

  M1  (STT[64]):  [PHI^{t+1} | X'] = ([V | S'] * dt) + [PHI | Z'']
  Q   (TS [32]):  q = i32( PHI^{t+1} * 1/2pi )          (round-to-nearest cast)
  R   (STT[32]):  r = (q * -2pi) + PHI^{t+1}            (range-reduced angle)
  D   (TT [32]):  [d | -d] = PHIswap - PHI              (swap via strided AP)
  P1,P2(STT[16]): p_c = (V_c * a_c) + X'_c
  NV  (STT[32]):  V^{t+1} = (D * dt*kappa) + P          (kappa1==kappa2 path)
  ACT:            S'^{t+1} = Sin(-r)  ( = -sin(phi^{t+1}) )

with a_c = 1 - dt*beta_c and host-prefolded noise Z'' = sigma_c*sqrt(dt)*z + dt*i_c.
Trajectory slots stream to DRAM in 128-step blocks; noise streams in the same way.
"""

import numpy as np

import concourse.bass as bass
import concourse.mybir as mybir
from concourse import bass_utils
from concourse.ap import AP

F32 = mybir.dt.float32
I32 = mybir.dt.int32
ALU = mybir.AluOpType
ACTF = mybir.ActivationFunctionType

N_CORES = 8
RS = 256          # phi/v/z ring slots (32 cols each)
KB = 128          # steps per DMA block (out and in)
SMALL = 4         # small ring slots for s'/x'/p/d/r/q

_CACHE = {}
LAST_RESULTS = None

TWO_PI = float(2.0 * np.pi)
INV_2PI = float(1.0 / (2.0 * np.pi))
RND_C = 12582912.0  # 1.5 * 2**23: (x + C) - C rounds x to nearest int in f32


def _build(dt, a1, a2, kp1, kp2, n_steps):
    """n_steps = N-1 update steps; emits the full unrolled program."""
    dt = float(dt)
    kap_equal = (kp1 == kp2)
    nc = bass.Bass()
    y0_in = nc.dram_tensor("y0_in", [128, 64], F32, kind="ExternalInput")
    z_in = nc.dram_tensor("z_in", [128, n_steps * 32], F32, kind="ExternalInput")
    phi_out = nc.dram_tensor("phi_out", [128, n_steps * 32], F32, kind="ExternalOutput")
    v_out = nc.dram_tensor("v_out", [128, n_steps * 32], F32, kind="ExternalOutput")

    n_blocks = (n_steps + KB - 1) // KB

    # ws column map (f32):
    PHI0 = 0                  # ring: RS*32
    V0 = PHI0 + RS * 32       # ring: RS*32
    Z0 = V0 + RS * 32         # ring: RS*32  (2 blocks of KB in flight)
    S0 = Z0 + RS * 32         # s' ring SMALL*32
    X0 = S0 + SMALL * 32      # x' ring
    P0 = X0 + SMALL * 32      # p ring
    D0 = P0 + SMALL * 32      # d ring
    R0 = D0 + SMALL * 32      # r ring
    Q0 = R0 + SMALL * 32      # rounded-q ring (f32)
    WCOLS = Q0 + SMALL * 32
    ROWS = WCOLS              # ws partition stride (elements)

    def wap(off, w):
        return AP(None, 0, [])  # placeholder; replaced below via closure

    with (
        nc.sbuf_tensor("ws", [128, WCOLS], F32) as ws,
        nc.semaphore("SY") as SY,
        nc.semaphore("SZ") as SZ,
        nc.semaphore("SV") as SV,
        nc.semaphore("SA") as SA,
        nc.semaphore("SOP") as SOP,
        nc.semaphore("SOV") as SOV,
        nc.Block() as block,
    ):
        def w1(off, w=32):
            # simple [128, w] view at column offset
            return AP(ws, off, [[ROWS, 128], [1, w]])

        def w2(off_a, off_b, w=32):
            # two-segment [128, 2, w] view: cols [off_a:off_a+w] ++ [off_b:off_b+w]
            return AP(ws, off_a, [[ROWS, 128], [off_b - off_a, 2], [1, w]])

        def wswap(off):
            # [128, 2, 16] reading halves swapped: [off+16:off+32] ++ [off:off+16]
            return AP(ws, off + 16, [[ROWS, 128], [-16, 2], [1, 16]])

        def w2d(off, w=32):
            # [128, 2, w] plain adjacent view (free-dims match the swap shape)
            return AP(ws, off, [[ROWS, 128], [w // 2, 2], [1, w // 2]])

        phi_at = lambda t: PHI0 + ((t - 1) % RS) * 32   # PHI^t lives at ring pos (t-1)%RS
        v_at = lambda t: V0 + ((t - 1) % RS) * 32
        z_at = lambda t: Z0 + (t % RS) * 32
        s_at = lambda t: S0 + (t % SMALL) * 32
        x_at = lambda t: X0 + (t % SMALL) * 32
        p_at = lambda t: P0 + (t % SMALL) * 32
        d_at = lambda t: D0 + (t % SMALL) * 32
        r_at = lambda t: R0 + (t % SMALL) * 32
        q_at = lambda t: Q0 + (t % SMALL) * 32

        @block.sync
        def _(sync):
            # prologue: y0 into ring slot for t=0, first two z blocks
            sync.dma_start(w1(phi_at(0)), y0_in[:, 0:32]).then_inc(SY, 16)
            sync.dma_start(w1(v_at(0)), y0_in[:, 32:64]).then_inc(SY, 16)
            for b in range(min(2, n_blocks)):
                lo, hi = b * KB, min(n_steps, (b + 1) * KB)
                sync.dma_start(
                    w1(Z0 + (lo % RS) * 32, (hi - lo) * 32),
                    z_in[:, lo * 32 : hi * 32],
                ).then_inc(SZ, 16)
            # streamed z loads + trajectory stores
            for b in range(2, n_blocks + 2):
                # store block b-2 once its last step retired
                sb = b - 2
                lo, hi = sb * KB, min(n_steps, (sb + 1) * KB)
                tE = hi - 1
                sync.wait_ge(SV, tE + 2)
                sync.dma_start(
                    phi_out[:, lo * 32 : hi * 32], w1(PHI0 + (lo % RS) * 32, (hi - lo) * 32)
                ).then_inc(SOP, 16)
                sync.dma_start(
                    v_out[:, lo * 32 : hi * 32], w1(V0 + (lo % RS) * 32, (hi - lo) * 32)
                ).then_inc(SOV, 16)
                if b < n_blocks:
                    lo2, hi2 = b * KB, min(n_steps, (b + 1) * KB)
                    # ring half reused by block b was consumed by block b-2's steps
                    sync.dma_start(
                        w1(Z0 + (lo2 % RS) * 32, (hi2 - lo2) * 32),
                        z_in[:, lo2 * 32 : hi2 * 32],
                    ).then_inc(SZ, 16)
            sync.wait_ge(SOP, 16 * n_blocks)
            sync.wait_ge(SOV, 16 * n_blocks)

        @block.vector
        def _(vector):
            vector.wait_ge(SY, 32)
            # prologue: q^0, r^0  (round-to-nearest via +C/-C trick, all f32)
            vector.tensor_scalar(w1(r_at(0)), w1(phi_at(0)), INV_2PI, RND_C, ALU.mult, ALU.add)
            vector.drain()
            vector.tensor_scalar(w1(q_at(0)), w1(r_at(0)), RND_C, None, ALU.subtract)
            vector.drain()
            vector.scalar_tensor_tensor(
                w1(r_at(0)), w1(q_at(0)), -TWO_PI, w1(phi_at(0)), ALU.mult, ALU.add
            )
            # R^0's visibility inc rides on D+-_0 below (pending-inc scheme):
            # an inc attached to the producing op itself races the write commit,
            # so every R^k's inc is carried by the first op of the next step.
            for t in range(n_steps):
                if t % KB == 0:
                    vector.wait_ge(SZ, 16 * (t // KB + 1))
                    if t >= RS:
                        # about to overwrite ring block (t//KB - 2); ensure drained
                        vector.wait_ge(SOP, 16 * (t // KB - 1))
                        vector.wait_ge(SOV, 16 * (t // KB - 1))
                # D: [d | -d] = PHIswap^t - PHI^t.  Carries the SV inc signalling
                # R^t committed (R^t retired >=1 op earlier); also spaces the SA
                # wait from M1's read of S'.
                vector.tensor_tensor(
                    w2d(d_at(t)), wswap(phi_at(t)), w2d(phi_at(t)), ALU.subtract
                ).then_inc(SV, 1)
                vector.wait_ge(SA, t + 1)
                # M1: [PHI^{t+1} | X'] = ([V^t | S'^t] * dt) + [PHI^t | Z''^t]
                vector.scalar_tensor_tensor(
                    w2(PHI0 + (t % RS) * 32, x_at(t)),
                    w2(v_at(t), s_at(t)),
                    dt,
                    w2(phi_at(t), z_at(t)),
                    ALU.mult,
                    ALU.add,
                )
                last = t == n_steps - 1
                # P1 (gap-1 after M1's X' write)
                vector.scalar_tensor_tensor(
                    w1(p_at(t), 16), w1(v_at(t), 16), a1, w1(x_at(t), 16),
                    ALU.mult, ALU.add,
                )
                if not last:
                    # QF = PHI^{t+1}*inv2pi + C   (gap-2 after M1)
                    vector.tensor_scalar(
                        w1(r_at(t + 1)), w1(PHI0 + (t % RS) * 32), INV_2PI, RND_C,
                        ALU.mult, ALU.add,
                    )
                # P2
                vector.scalar_tensor_tensor(
                    w1(p_at(t) + 16, 16), w1(v_at(t) + 16, 16), a2, w1(x_at(t) + 16, 16),
                    ALU.mult, ALU.add,
                )
                if not last:
                    # Q = QF - C   (gap-1 after QF)
                    vector.tensor_scalar(
                        w1(q_at(t + 1)), w1(r_at(t + 1)), RND_C, None, ALU.subtract
                    )
                # NV (>=2 ops after D/P1/P2)
                if kap_equal:
                    vector.scalar_tensor_tensor(
                        w1(V0 + (t % RS) * 32), w1(d_at(t)), dt * kp1, w1(p_at(t)),
                        ALU.mult, ALU.add,
                    )
                else:
                    vector.scalar_tensor_tensor(
                        w1(V0 + (t % RS) * 32, 16), w1(d_at(t), 16), dt * kp1,
                        w1(p_at(t), 16), ALU.mult, ALU.add,
                    )
                    vector.scalar_tensor_tensor(
                        w1(V0 + (t % RS) * 32 + 16, 16), w1(d_at(t) + 16, 16), dt * kp2,
                        w1(p_at(t) + 16, 16), ALU.mult, ALU.add,
                    )
                if not last:
                    # R = Q*-2pi + PHI^{t+1}  (gap-1 after Q via NV); its
                    # visibility inc rides on D+-_{t+1}.
                    vector.scalar_tensor_tensor(
                        w1(r_at(t + 1)),
                        w1(q_at(t + 1)),
                        -TWO_PI,
                        w1(PHI0 + (t % RS) * 32),
                        ALU.mult,
                        ALU.add,
                    )
            # final: one drain to commit the tail (NV of the last step) for DMA
            vector.drain().then_inc(SV, 1)

        @block.scalar
        def _(scalar):
            for k in range(n_steps):
                scalar.wait_ge(SV, k + 1)
                scalar.activation(w1(s_at(k)), w1(r_at(k)), ACTF.Sin, scale=-1.0).then_inc(SA, 1)

    return nc


def _prep_inputs(params, y0, noise, dt):
    """Host-side: shard + relayout.  Returns per-core in_maps."""
    B = y0.shape[0]
    bpc = B // N_CORES
    n_steps = noise.shape[0]
    sq = np.float32(np.sqrt(dt))
    sig = params[6:8].astype(np.float32)
    drive = params[2:4].astype(np.float32)
    scale = (sig * sq).reshape(1, 1, 2, 1)          # per comp
    off = (drive * np.float32(dt)).reshape(1, 1, 2, 1)

    in_maps = []
    for c in range(N_CORES):
        sl = slice(c * bpc, (c + 1) * bpc)
        # z'': [n_steps, bpc, 2] -> [128, n_steps*32] with cols t*32 + comp*16 + j
        zc = noise[:, sl, :].reshape(n_steps, 128, 16, 2).transpose(1, 0, 3, 2)
        zc = zc * scale + off                        # [128, n_steps, 2, 16]
        zc = np.ascontiguousarray(zc.reshape(128, n_steps * 32), dtype=np.float32)
        y0c = y0[sl].reshape(128, 16, 4)
        y0lay = np.empty((128, 64), np.float32)
        y0lay[:, 0:16] = y0c[:, :, 0]    # phi1
        y0lay[:, 16:32] = y0c[:, :, 2]   # phi2
        y0lay[:, 32:48] = y0c[:, :, 1]   # v1
        y0lay[:, 48:64] = y0c[:, :, 3]   # v2
        in_maps.append({"y0_in": y0lay, "z_in": zc})
    return in_maps


def _assemble(results, y0, n_steps):
    B = y0.shape[0]
    bpc = B // N_CORES
    out = np.empty((B, n_steps + 1, 4), np.float32)
    out[:, 0, :] = y0
    for c in range(N_CORES):
        sl = slice(c * bpc, (c + 1) * bpc)
        ph = results[c]["phi_out"].reshape(128, n_steps, 2, 16)
        vv = results[c]["v_out"].reshape(128, n_steps, 2, 16)
        # batch b = p*16 + j ; comp axis -> state cols
        blk = out[sl, 1:, :].reshape(128, 16, n_steps, 4)
        blk[:, :, :, 0] = ph[:, :, 0, :].transpose(0, 2, 1)
        blk[:, :, :, 2] = ph[:, :, 1, :].transpose(0, 2, 1)
        blk[:, :, :, 1] = vv[:, :, 0, :].transpose(0, 2, 1)
        blk[:, :, :, 3] = vv[:, :, 1, :].transpose(0, 2, 1)
        out[sl, 1:, :] = blk.reshape(bpc, n_steps, 4)
    return out


def kernel(params, y0, noise, T, N):
    global LAST_RESULTS
    params = np.asarray(params, dtype=np.float32)
    y0 = np.asarray(y0, dtype=np.float32)
    noise = np.asarray(noise, dtype=np.float32)
    N = int(N)
    n_steps = N - 1
    assert noise.shape[0] == n_steps
    dt = np.float32(T) / np.float32(N - 1)
    beta1, beta2 = float(params[0]), float(params[1])
    kp1, kp2 = float(params[4]), float(params[5])
    a1 = float(np.float32(1.0) - dt * np.float32(beta1))
    a2 = float(np.float32(1.0) - dt * np.float32(beta2))

    key = (float(dt), a1, a2, kp1, kp2, n_steps)
    if key not in _CACHE:
        _CACHE[key] = _build(dt, a1, a2, kp1, kp2, n_steps)
    nc = _CACHE[key]

    in_maps = _prep_inputs(params, y0, noise, float(dt))
    res = bass_utils.run_bass_kernel_spmd(nc, in_maps, core_ids=list(range(N_CORES)))
    LAST_RESULTS = res
    return _assemble(res.results, y0, n_steps)


# revision 12
# speedup vs baseline: 1.1217x; 1.0762x over previous
"""Josephson-junction SDE: Euler-Maruyama fully on-device, batch-sharded 8 ways.

Per core: batch 2048, state packed as [128 part, 32 cols] (= 2048 batch x 2
components on free-dim halves).  Per step t (f32 throughout):

  M1  (STT[64]):  [PHI^{t+1} | X'] = ([V | S'] * dt) + [PHI | Z'']
  Q   (TS [32]):  q = i32( PHI^{t+1} * 1/2pi )          (round-to-nearest cast)
  R   (STT[32]):  r = (q * -2pi) + PHI^{t+1}            (range-reduced angle)
  D   (TT [32]):  [d | -d] = PHIswap - PHI              (swap via strided AP)
  P1,P2(STT[16]): p_c = (V_c * a_c) + X'_c
  NV  (STT[32]):  V^{t+1} = (D * dt*kappa) + P          (kappa1==kappa2 path)
  ACT:            S'^{t+1} = Sin(-r)  ( = -sin(phi^{t+1}) )

with a_c = 1 - dt*beta_c and host-prefolded noise Z'' = sigma_c*sqrt(dt)*z + dt*i_c.
Trajectory slots stream to DRAM in 128-step blocks; noise streams in the same way.
"""

import numpy as np

import concourse.bass as bass
import concourse.mybir as mybir
from concourse import bass_utils
from concourse.ap import AP

F32 = mybir.dt.float32
I32 = mybir.dt.int32
ALU = mybir.AluOpType
ACTF = mybir.ActivationFunctionType

N_CORES = 8
RS = 256          # phi/v/z ring slots (32 cols each)
KB = 128          # steps per DMA block (out and in)
SMALL = 4         # small ring slots for s'/x'/p/d/r/q

_CACHE = {}
LAST_RESULTS = None

TWO_PI = float(2.0 * np.pi)
INV_2PI = float(1.0 / (2.0 * np.pi))
RND_C = 12582912.0  # 1.5 * 2**23: (x + C) - C rounds x to nearest int in f32


def _build(dt, a1, a2, kp1, kp2, n_steps):
    """n_steps = N-1 update steps; emits the full unrolled program."""
    dt = float(dt)
    kap_equal = (kp1 == kp2)
    nc = bass.Bass()
    y0_in = nc.dram_tensor("y0_in", [128, 64], F32, kind="ExternalInput")
    z_in = nc.dram_tensor("z_in", [128, n_steps * 32], F32, kind="ExternalInput")
    phi_out = nc.dram_tensor("phi_out", [128, n_steps * 32], F32, kind="ExternalOutput")
    v_out = nc.dram_tensor("v_out", [128, n_steps * 32], F32, kind="ExternalOutput")

    n_blocks = (n_steps + KB - 1) // KB

    # ws column map (f32):
    PHI0 = 0                  # ring: RS*32
    V0 = PHI0 + RS * 32       # ring: RS*32
    Z0 = V0 + RS * 32         # ring: RS*32  (2 blocks of KB in flight)
    S0 = Z0 + RS * 32         # s' ring SMALL*32
    X0 = S0 + SMALL * 32      # x' ring
    P0 = X0 + SMALL * 32      # p ring
    D0 = P0 + SMALL * 32      # d ring
    R0 = D0 + SMALL * 32      # r ring
    Q0 = R0 + SMALL * 32      # rounded-q ring (f32)
    PS0 = Q0 + SMALL * 32     # scaled-phi state ring (phi/2pi, resynced)
    YC0 = PS0 + SMALL * 32    # Dekker temp ring
    WCOLS = YC0 + SMALL * 32
    ROWS = WCOLS              # ws partition stride (elements)

    def wap(off, w):
        return AP(None, 0, [])  # placeholder; replaced below via closure

    with (
        nc.sbuf_tensor("ws", [128, WCOLS], F32) as ws,
        nc.semaphore("SY") as SY,
        nc.semaphore("SZ") as SZ,
        nc.semaphore("SV") as SV,
        nc.semaphore("SA") as SA,
        nc.semaphore("SOP") as SOP,
        nc.semaphore("SOV") as SOV,
        nc.Block() as block,
    ):
        def w1(off, w=32):
            # simple [128, w] view at column offset
            return AP(ws, off, [[ROWS, 128], [1, w]])

        def w2(off_a, off_b, w=32):
            # two-segment [128, 2, w] view: cols [off_a:off_a+w] ++ [off_b:off_b+w]
            return AP(ws, off_a, [[ROWS, 128], [off_b - off_a, 2], [1, w]])

        def wswap(off):
            # [128, 2, 16] reading halves swapped: [off+16:off+32] ++ [off:off+16]
            return AP(ws, off + 16, [[ROWS, 128], [-16, 2], [1, 16]])

        def w2d(off, w=32):
            # [128, 2, w] plain adjacent view (free-dims match the swap shape)
            return AP(ws, off, [[ROWS, 128], [w // 2, 2], [1, w // 2]])

        phi_at = lambda t: PHI0 + ((t - 1) % RS) * 32   # PHI^t lives at ring pos (t-1)%RS
        v_at = lambda t: V0 + ((t - 1) % RS) * 32
        z_at = lambda t: Z0 + (t % RS) * 32
        s_at = lambda t: S0 + (t % SMALL) * 32
        x_at = lambda t: X0 + (t % SMALL) * 32
        p_at = lambda t: P0 + (t % SMALL) * 32
        d_at = lambda t: D0 + (t % SMALL) * 32
        r_at = lambda t: R0 + (t % SMALL) * 32
        q_at = lambda t: Q0 + (t % SMALL) * 32
        ps_at = lambda t: PS0 + (t % SMALL) * 32
        yc_at = lambda t: YC0 + (t % SMALL) * 32

        @block.sync
        def _(sync):
            # prologue: y0 into ring slot for t=0, first two z blocks
            sync.dma_start(w1(phi_at(0)), y0_in[:, 0:32]).then_inc(SY, 16)
            sync.dma_start(w1(v_at(0)), y0_in[:, 32:64]).then_inc(SY, 16)
            for b in range(min(2, n_blocks)):
                lo, hi = b * KB, min(n_steps, (b + 1) * KB)
                sync.dma_start(
                    w1(Z0 + (lo % RS) * 32, (hi - lo) * 32),
                    z_in[:, lo * 32 : hi * 32],
                ).then_inc(SZ, 16)
            # streamed z loads + trajectory stores
            for b in range(2, n_blocks + 2):
                # store block b-2 once its last step retired
                sb = b - 2
                lo, hi = sb * KB, min(n_steps, (sb + 1) * KB)
                tE = hi - 1
                if tE == n_steps - 1:
                    sync.wait_ge(SV, n_steps + 2)
                else:
                    sync.wait_ge(SV, tE + 2)
                sync.dma_start(
                    phi_out[:, lo * 32 : hi * 32], w1(PHI0 + (lo % RS) * 32, (hi - lo) * 32)
                ).then_inc(SOP, 16)
                sync.dma_start(
                    v_out[:, lo * 32 : hi * 32], w1(V0 + (lo % RS) * 32, (hi - lo) * 32)
                ).then_inc(SOV, 16)
                if b < n_blocks:
                    lo2, hi2 = b * KB, min(n_steps, (b + 1) * KB)
                    # ring half reused by block b was consumed by block b-2's steps
                    sync.dma_start(
                        w1(Z0 + (lo2 % RS) * 32, (hi2 - lo2) * 32),
                        z_in[:, lo2 * 32 : hi2 * 32],
                    ).then_inc(SZ, 16)
            sync.wait_ge(SOP, 16 * n_blocks)
            sync.wait_ge(SOV, 16 * n_blocks)

        @block.vector
        def _(vector):
            DTI = dt * INV_2PI
            RSYNC = 8
            vector.wait_ge(SY, 32)
            # prologue: PHIS^0 = PHI^0/2pi, F^0 = PHIS^0 - round(PHIS^0)
            vector.tensor_scalar(w1(ps_at(0)), w1(phi_at(0)), INV_2PI, None, ALU.mult)
            vector.drain()
            vector.tensor_scalar(w1(yc_at(0)), w1(ps_at(0)), RND_C, None, ALU.add)
            vector.drain()
            vector.tensor_scalar(w1(q_at(0)), w1(yc_at(0)), RND_C, None, ALU.subtract)
            vector.drain()
            vector.scalar_tensor_tensor(
                w1(r_at(0)), w1(q_at(0)), -1.0, w1(ps_at(0)), ALU.mult, ALU.add
            )
            vector.drain().then_inc(SV, 1)
            for t in range(n_steps):
                if t % KB == 0:
                    vector.wait_ge(SZ, 16 * (t // KB + 1))
                    if t >= RS:
                        vector.wait_ge(SOP, 16 * (t // KB - 1))
                        vector.wait_ge(SOV, 16 * (t // KB - 1))
                if t > 0 and t % RSYNC == 0:
                    # resync the scaled-phi state from true phi (kills drift);
                    # PHI^t has been committed since step t-1.
                    vector.tensor_scalar(
                        w1(ps_at(t)), w1(phi_at(t)), INV_2PI, None, ALU.mult
                    )
                # D: [d | -d] = PHIswap^t - PHI^t  (also the gap for PHIS's read
                # of V^t written by NV_{t-1})
                vector.tensor_tensor(
                    w2d(d_at(t)), wswap(phi_at(t)), w2d(phi_at(t)), ALU.subtract
                )
                last = t == n_steps - 1
                if not last:
                    # PHIS^{t+1} = (V^t * dt/2pi) + PHIS^t
                    vector.scalar_tensor_tensor(
                        w1(ps_at(t + 1)), w1(v_at(t)), DTI, w1(ps_at(t)),
                        ALU.mult, ALU.add,
                    )
                vector.wait_ge(SA, t + 1)
                # M1: [PHI^{t+1} | X'] = ([V^t | S'^t] * dt) + [PHI^t | Z''^t]
                vector.scalar_tensor_tensor(
                    w2(PHI0 + (t % RS) * 32, x_at(t)),
                    w2(v_at(t), s_at(t)),
                    dt,
                    w2(phi_at(t), z_at(t)),
                    ALU.mult,
                    ALU.add,
                )
                if not last:
                    # YC = PHIS^{t+1} + C   (Dekker round-to-nearest, part 1)
                    vector.tensor_scalar(
                        w1(yc_at(t + 1)), w1(ps_at(t + 1)), RND_C, None, ALU.add
                    )
                # P1
                vector.scalar_tensor_tensor(
                    w1(p_at(t), 16), w1(v_at(t), 16), a1, w1(x_at(t), 16),
                    ALU.mult, ALU.add,
                )
                if not last:
                    # Qd = YC - C
                    vector.tensor_scalar(
                        w1(q_at(t + 1)), w1(yc_at(t + 1)), RND_C, None, ALU.subtract
                    )
                # P2
                vector.scalar_tensor_tensor(
                    w1(p_at(t) + 16, 16), w1(v_at(t) + 16, 16), a2, w1(x_at(t) + 16, 16),
                    ALU.mult, ALU.add,
                )
                if not last:
                    # F = PHIS^{t+1} - Qd  in [-0.5, 0.5]; ACT computes
                    # sin(-2pi*F) = -sin(phi^{t+1})
                    vector.scalar_tensor_tensor(
                        w1(r_at(t + 1)), w1(q_at(t + 1)), -1.0, w1(ps_at(t + 1)),
                        ALU.mult, ALU.add,
                    )
                # NV; carries the SV inc (1 op after F -> F committed for ACT)
                if kap_equal:
                    vector.scalar_tensor_tensor(
                        w1(V0 + (t % RS) * 32), w1(d_at(t)), dt * kp1, w1(p_at(t)),
                        ALU.mult, ALU.add,
                    ).then_inc(SV, 1)
                else:
                    vector.scalar_tensor_tensor(
                        w1(V0 + (t % RS) * 32, 16), w1(d_at(t), 16), dt * kp1,
                        w1(p_at(t), 16), ALU.mult, ALU.add,
                    )
                    vector.scalar_tensor_tensor(
                        w1(V0 + (t % RS) * 32 + 16, 16), w1(d_at(t) + 16, 16), dt * kp2,
                        w1(p_at(t) + 16, 16), ALU.mult, ALU.add,
                    ).then_inc(SV, 1)
            # final: commit the tail for the last DMA block
            vector.drain().then_inc(SV, 1)

        @block.scalar
        def _(scalar):
            for k in range(n_steps):
                scalar.wait_ge(SV, k + 1)
                scalar.activation(w1(s_at(k)), w1(r_at(k)), ACTF.Sin, scale=-TWO_PI).then_inc(SA, 1)

    return nc


def _prep_inputs(params, y0, noise, dt):
    """Host-side: shard + relayout.  Returns per-core in_maps."""
    B = y0.shape[0]
    bpc = B // N_CORES
    n_steps = noise.shape[0]
    sq = np.float32(np.sqrt(dt))
    sig = params[6:8].astype(np.float32)
    drive = params[2:4].astype(np.float32)
    scale = (sig * sq).reshape(1, 1, 2, 1)          # per comp
    off = (drive * np.float32(dt)).reshape(1, 1, 2, 1)

    in_maps = []
    for c in range(N_CORES):
        sl = slice(c * bpc, (c + 1) * bpc)
        # z'': [n_steps, bpc, 2] -> [128, n_steps*32] with cols t*32 + comp*16 + j
        zc = noise[:, sl, :].reshape(n_steps, 128, 16, 2).transpose(1, 0, 3, 2)
        zc = zc * scale + off                        # [128, n_steps, 2, 16]
        zc = np.ascontiguousarray(zc.reshape(128, n_steps * 32), dtype=np.float32)
        y0c = y0[sl].reshape(128, 16, 4)
        y0lay = np.empty((128, 64), np.float32)
        y0lay[:, 0:16] = y0c[:, :, 0]    # phi1
        y0lay[:, 16:32] = y0c[:, :, 2]   # phi2
        y0lay[:, 32:48] = y0c[:, :, 1]   # v1
        y0lay[:, 48:64] = y0c[:, :, 3]   # v2
        in_maps.append({"y0_in": y0lay, "z_in": zc})
    return in_maps


def _assemble(results, y0, n_steps):
    B = y0.shape[0]
    bpc = B // N_CORES
    out = np.empty((B, n_steps + 1, 4), np.float32)
    out[:, 0, :] = y0
    for c in range(N_CORES):
        sl = slice(c * bpc, (c + 1) * bpc)
        ph = results[c]["phi_out"].reshape(128, n_steps, 2, 16)
        vv = results[c]["v_out"].reshape(128, n_steps, 2, 16)
        # batch b = p*16 + j ; comp axis -> state cols
        blk = out[sl, 1:, :].reshape(128, 16, n_steps, 4)
        blk[:, :, :, 0] = ph[:, :, 0, :].transpose(0, 2, 1)
        blk[:, :, :, 2] = ph[:, :, 1, :].transpose(0, 2, 1)
        blk[:, :, :, 1] = vv[:, :, 0, :].transpose(0, 2, 1)
        blk[:, :, :, 3] = vv[:, :, 1, :].transpose(0, 2, 1)
        out[sl, 1:, :] = blk.reshape(bpc, n_steps, 4)
    return out


def kernel(params, y0, noise, T, N):
    global LAST_RESULTS
    params = np.asarray(params, dtype=np.float32)
    y0 = np.asarray(y0, dtype=np.float32)
    noise = np.asarray(noise, dtype=np.float32)
    N = int(N)
    n_steps = N - 1
    assert noise.shape[0] == n_steps
    dt = np.float32(T) / np.float32(N - 1)
    beta1, beta2 = float(params[0]), float(params[1])
    kp1, kp2 = float(params[4]), float(params[5])
    a1 = float(np.float32(1.0) - dt * np.float32(beta1))
    a2 = float(np.float32(1.0) - dt * np.float32(beta2))

    key = (float(dt), a1, a2, kp1, kp2, n_steps)
    if key not in _CACHE:
        _CACHE[key] = _build(dt, a1, a2, kp1, kp2, n_steps)
    nc = _CACHE[key]

    in_maps = _prep_inputs(params, y0, noise, float(dt))
    res = bass_utils.run_bass_kernel_spmd(nc, in_maps, core_ids=list(range(N_CORES)))
    LAST_RESULTS = res
    return _assemble(res.results, y0, n_steps)
